# revision 41
# baseline (speedup 1.0000x reference)
"""CircleLossV2 on 8 Trainium2 NeuronCores (Bass/Tile) — symmetric triangle v10.

60.4us HW exec (vs 83.7us baseline), rel err 1.2e-6. Each group's first
512-col piece is prefetched into a dedicated PSUM bank at the end of the
previous group's step so the DVE never stalls at group boundaries.

Strategy (rebuilt from the 74us baseline):
  - Host: sort rows by label; per-core rotate by 512*k cols so each core owns
    tile-rows {0..3, 32..35} of its rotated copy (SPMD). sim is symmetric:
    tile-row i computes tiles [i, i+W) (W=33 for i<32 else 32) — every
    unordered pair once. Host normalizes in fp64, ships eT extended layout
    [128, 8704] bf16 (left pad 128, wrap).
  - Device per row-group: PE bf16 matmuls -> fp32 PSUM sim in 1536-col pieces
    (diag killed by a -2I matmul); ONE fused custom-DVE op per piece
    v=(s+0.75)^2 (sq(Src0+C0), registered via the framework's custom-DVE
    extension API) evacuates PSUM->SBUF fp32 in a single pass; a small
    ACT-Square share keeps DVE/ACT balanced; one big ACT Exp per group
    (bias -140, scale 64) -> E bf16 with accum_out row sums; column sums via
    one-hot-column matmuls into a single pre-zeroed [18,512] fp32 PSUM tile.
  - pw windows for the host pos/CR terms are v[:, 0:192] slices (fp32 u^2,
    exactly what the device exponentiates), DMA'd per group — no extra
    matmuls or copies.
  - No all_engine_barrier and no gpsimd anywhere: avoids serializing on the
    ~6us Q7 boot. Consts are framework-tracked tiles memset by the DVE.
    xt arrives as 4 large range-ordered dma_starts so compute starts ~2us in.
  - Host epilogue: row slots + flat colsums; right-sided same-class window
    (each unordered pair once, added to both members); fp64 recompute
    fallback for cancellation rows.
"""

import sys

sys.path.insert(0, "/opt/trn_rl_repo")

import numpy as np
from ml_dtypes import bfloat16

import concourse.bass as bass
import concourse.bacc as bacc
import concourse.mybir as mybir
import concourse.tile as tile
import concourse.dve_ops as dve_ops
from concourse.dve_spec import Spec, Src0, C0, sq, lower as dve_lower
from concourse.dve_spec import _has_src1 as dve_has_src1
from concourse.dve_uop import DveOpSpec
from concourse.bass_utils import run_bass_kernel_spmd

F32 = mybir.dt.float32
F16 = mybir.dt.float16
BF16 = mybir.dt.bfloat16
AF = mybir.ActivationFunctionType
OP = mybir.AluOpType

B = 8192
D = 128
NCORES = 8
EXT_OFF = 128
EXT_W = 8704
MHN = 140.0
MHP = 100.0
LOCAL_ROWS = [0, 1, 2, 3, 32, 33, 34, 35]
NSLOT = 13

_PROG = None


def _sqadd_op():
    """Register (once) and return the fused v=(s+c)^2 custom-DVE op."""
    name = "CIRCLE_SQADD_ANT"
    for o in dve_ops.OPS:
        if o.name == name:
            return o
    spec = Spec(
        body=sq(Src0 + C0),
        reference=lambda in0, in1, s0, s1, imm2: (
            (in0.astype(np.float32) + s0) ** 2
        ),
    )
    row = max(dve_ops._SUB_OPCODE_FOR_NAME.values()) + 1
    assert row < 0x20
    dve_ops._SUB_OPCODE_FOR_NAME[name] = row
    shas = {}
    for ver in ("v3",):
        u = dve_lower(spec, ver=ver)
        shas[ver] = DveOpSpec(
            name=name, opcode=row, uops=u, rd1_en=dve_has_src1(spec)
        ).sha(ver)
    op = dve_ops.DveOp(name, spec, subdim=False, uops_sha=shas)
    dve_ops.OPS.append(op)
    dve_ops.CUSTOM_DVE_SPECS[name] = spec
    return op


SQADD = _sqadd_op()


def _wtiles(i):
    return 33 if i < 32 else 32


def _build_schedule():
    groups = []
    for r, i in enumerate(LOCAL_ROWS):
        lo = EXT_OFF + i * 128
        wid = _wtiles(i) * 128
        # pieces: (piece_a, piece_b, act_tail_cols). Piece 0 is a small
        # 512-col prefetch piece computed in a dedicated PSUM bank at the
        # end of the PREVIOUS group's step, so the DVE never stalls at
        # group boundaries.
        pieces = [(0, 512, 0), (512, 1536, 256), (1536, 3072, 0),
                  (3072, wid, 0)]
        spans = []
        a, b = lo + 128, lo + wid
        while a < b:
            nb = min(b, 512 * (a // 512 + 1))
            spans.append((a // 512, a, nb))
            a = nb
        if r == 0:
            segs = [(0, 1536, 0), (1536, 3072, 8), (3072, wid, 10)]
        elif r == 7:
            segs = [(0, 1536, 7), (1536, 3072, 9), (3072, wid, 11)]
        else:
            segs = [(0, wid, r)]
        groups.append(dict(r=r, i=i, lo=lo, wid=wid, pieces=pieces,
                           spans=spans, segs=segs))
    return groups


GROUPS = _build_schedule()
NGRP = len(GROUPS)


def _build():
    nc = bacc.Bacc("TRN2", target_bir_lowering=False, debug=False,
                   num_devices=NCORES)

    xt_in = nc.dram_tensor("xt", [D, EXT_W], BF16, kind="ExternalInput")
    cst_in = nc.dram_tensor("cst", [128, 292], BF16, kind="ExternalInput")
    stats_out = nc.dram_tensor("stats", [128, NSLOT], F32, kind="ExternalOutput")
    cs_out = nc.dram_tensor("cs", [18, 512], F32, kind="ExternalOutput")
    pw_out = nc.dram_tensor("pw", [128, 1536], F32, kind="ExternalOutput")

    with tile.TileContext(nc) as tc:
        with (
            tc.tile_pool(name="cst", bufs=1) as cst,
            tc.tile_pool(name="sbv", bufs=3) as sbv,   # fp32 v = (s+0.75)^2
            tc.tile_pool(name="sbe", bufs=3) as sbe,   # bf16 E
            tc.tile_pool(name="psd", bufs=2, space="PSUM") as psd,
            tc.tile_pool(name="psx", bufs=1, space="PSUM") as psx,
            tc.tile_pool(name="psc", bufs=1, space="PSUM") as psc,
        ):
            # ---------------- inputs (range-ordered, few large DMAs) -------
            xt = cst.tile([128, EXT_W], BF16, tag="xt", name="xt")
            XT_CUTS = [0, 1664, 3584, 5760, EXT_W]
            nc.sync.dma_start(xt[:, 0:1664], xt_in.ap()[:, 0:1664])

            consts = cst.tile([128, 292], BF16, tag="consts", name="consts")
            nc.sync.dma_start(consts[:], cst_in.ap())
            i128 = consts[:, 0:128]
            n2i = consts[:, 128:256]
            onehot = consts[:, 256:291]  # rel col 17 is all-ones

            for c0, c1 in zip(XT_CUTS[1:-1], XT_CUTS[2:]):
                nc.sync.dma_start(xt[:, c0:c1], xt_in.ap()[:, c0:c1])

            # consts as framework-tracked tiles (no gpsimd, no barrier)
            cb = cst.tile([128, 2], F32, tag="cb", name="cb")
            nc.vector.memset(cb[:, 0:1], -MHN)
            nc.vector.memset(cb[:, 1:2], 0.75)
            nc.const_aps.aps[(F32, -MHN)] = cb[:, 0:1]
            nc.const_aps.aps[(F32, 0.75)] = cb[:, 1:2]

            NS = cst.tile([128, NSLOT], F32, tag="NS", name="NS")
            css = cst.tile([18, 512], F32, tag="css", name="css")

            # hoist ACT table loads to t=0
            warm0 = cst.tile([128, 1], F32, tag="warm0", name="warm0")
            nc.vector.memset(warm0[:], 0.0)
            warm1 = cst.tile([128, 1], F32, tag="warm1", name="warm1")
            nc.scalar.activation(warm1[:], warm0[:], AF.Exp,
                                 bias=-MHN, scale=64.0)
            nc.scalar.activation(warm1[:], warm0[:], AF.Square, bias=0.75)

            # pre-zero the colsum accumulator (all colsum matmuls accumulate)
            csP = psc.tile([18, 512], F32, tag="csP", name="csP")
            nc.vector.memset(csP[:], 0.0)

            state = {}

            def emit_cs_spans(gprev, spans, is_last_g):
                # a few colsum spans of an earlier group, interleaved between
                # main pieces so PE colsum bursts never block the psd refill
                E = state[("E", gprev["r"])]
                lo = gprev["lo"]
                nsp = len(gprev["spans"])
                for si, (h, aa, bb) in spans:
                    nc.tensor.matmul(
                        csP[0:18, aa - 512 * h: bb - 512 * h],
                        onehot[:, 17 - h: 35 - h],
                        E[:, aa - lo: bb - lo],
                        start=False,
                        stop=is_last_g and si == nsp - 1,
                        skip_group_check=True,
                    )

            def emit_piece(g, pi):
                lo, r = g["lo"], g["r"]
                pa, pb, act_w = g["pieces"][pi]
                w = pb - pa
                if pi == 0:
                    v = sbv.tile([128, 4224], F32, tag="v", name=f"v{r}")
                    state[("v", r)] = v
                v = state[("v", r)]
                if pi == 0:
                    ps_t = psx.tile([128, 512], F32, tag="px",
                                    name=f"px_{r}")
                else:
                    ps_t = psd.tile([128, 1536], F32, tag="ps",
                                    name=f"ps_{r}_{pa}")
                lhsT = xt[:, lo: lo + 128]
                for s0 in range(pa, pb, 512):
                    s1 = min(s0 + 512, pb)
                    has_diag = s0 == 0
                    nc.tensor.matmul(
                        ps_t[:, s0 - pa: s1 - pa], lhsT,
                        xt[:, lo + s0: lo + s1],
                        start=True, stop=not has_diag,
                    )
                    if has_diag:
                        nc.tensor.matmul(
                            ps_t[:, 0:128], n2i, i128,
                            start=False, stop=True, skip_group_check=True,
                        )
                dw = w - act_w
                nc.vector._custom_dve(
                    SQADD, out=v[:, pa:pa + dw], in0=ps_t[:, 0:dw], s0=0.75,
                )
                if act_w:
                    nc.scalar.activation(
                        v[:, pa + dw:pb], ps_t[:, dw:w], AF.Square, bias=0.75,
                    )
                if pi == 0:
                    # window slice for host pos/CR terms (v = (s+0.75)^2)
                    nc.sync.dma_start(
                        pw_out.ap()[:, r * 192:(r + 1) * 192], v[:, 0:192],
                    )

            def emit_exp(g):
                v = state.pop(("v", g["r"]))
                E = sbe.tile([128, 4224], BF16, tag="E", name=f"E{g['r']}")
                state[("E", g["r"])] = E
                for (a, b, slot) in g["segs"]:
                    nc.scalar.activation(
                        E[:, a:b], v[:, a:b], AF.Exp,
                        bias=-MHN, scale=64.0,
                        accum_out=NS[:, slot: slot + 1],
                    )

            emit_piece(GROUPS[0], 0)
            for step in range(NGRP + 2):
                g = GROUPS[step] if step < NGRP else None
                gprev = GROUPS[step - 2] if step >= 2 else None
                is_last_g = step - 2 == NGRP - 1
                # exp of the previous group FIRST on the ACT queue — its
                # input is already complete, while this step's A-share
                # square is not (avoids in-order head-of-line blocking)
                if 1 <= step < NGRP + 1:
                    emit_exp(GROUPS[step - 1])
                spans = (list(enumerate(gprev["spans"]))
                         if gprev is not None else [])
                third = (len(spans) + 2) // 3
                if g is not None:
                    for pi in range(1, len(g["pieces"])):
                        emit_piece(g, pi)
                        if pi >= 2 and spans:
                            take, spans = spans[:third], spans[third:]
                            emit_cs_spans(gprev, take, is_last_g)
                    if step + 1 < NGRP:
                        emit_piece(GROUPS[step + 1], 0)
                if spans:
                    emit_cs_spans(gprev, spans, is_last_g)
                if gprev is not None:
                    state.pop(("E", gprev["r"]))

            nc.sync.dma_start(stats_out.ap(), NS[:])
            nc.vector.tensor_copy(css[:], csP[:])
            nc.scalar.dma_start(cs_out.ap(), css[:])

    nc.compile()
    return nc


def _get_prog():
    global _PROG
    if _PROG is None:
        _PROG = _build()
    return _PROG


def _prepare_inputs(embeddings, labels):
    x = np.asarray(embeddings, dtype=np.float32)
    lab = np.asarray(labels)
    assert x.shape == (B, D) and lab.shape == (B,)

    perm = np.argsort(lab, kind="stable")
    xs = x[perm]
    ls = lab[perm]

    _, inv_idx, counts = np.unique(ls, return_inverse=True, return_counts=True)
    cnt_row = counts[inv_idx]
    valid_sorted = (cnt_row >= 2) & (B - cnt_row >= 1)
    assert counts.max() <= 64, "window of 192 requires class size <= 64"

    e64 = xs.astype(np.float64)
    e64 /= np.linalg.norm(e64, axis=1, keepdims=True)
    e = e64.astype(np.float32)
    eT = np.ascontiguousarray(e.T)

    cst = np.zeros((128, 292), dtype=bfloat16)
    cst[:, 0:128] = np.eye(128, dtype=bfloat16)
    cst[:, 128:256] = (-2.0 * np.eye(128)).astype(bfloat16)
    cst[:, 256 + 17] = 1.0

    ext_src = (np.arange(EXT_W) - EXT_OFF) % B
    in_maps = []
    for k in range(NCORES):
        sh = 512 * k
        rot_cols = (ext_src + sh) % B
        xt = np.ascontiguousarray(eT[:, rot_cols]).astype(bfloat16)
        in_maps.append({"xt": xt, "cst": cst})
    return in_maps, valid_sorted, ls, e64


def _epilogue(results, valid_sorted, ls, e64):
    NEG = np.zeros(B)
    CRv = np.zeros(B)
    PSv = np.zeros(B)
    prow = np.arange(128)
    ext_idx = np.arange(EXT_W) - EXT_OFF
    win = np.arange(192)
    triu = win[None, :] > prow[:, None]  # strict upper: each pair once
    for k in range(NCORES):
        st = np.asarray(results[k]["stats"], dtype=np.float64)
        cs = np.asarray(results[k]["cs"], dtype=np.float64)
        pwf = np.asarray(results[k]["pw"], dtype=np.float64)
        sh = 512 * k
        for g in GROUPS:
            rows = (sh + g["i"] * 128 + prow) % B
            for (_, _, slot) in g["segs"]:
                NEG[rows] += st[:, slot]
        csf = cs.reshape(-1)[:EXT_W]
        np.add.at(NEG, (ext_idx + sh) % B, csf)
        for r_idx, i in enumerate(LOCAL_ROWS):
            rows = (sh + i * 128 + prow) % B
            wcols = (sh + i * 128 + win) % B
            eq = (ls[rows][:, None] == ls[wcols][None, :]) & triu
            v = pwf[:, r_idx * 192:(r_idx + 1) * 192]  # (s+0.75)^2 fp32
            u = np.sqrt(np.maximum(v, 0.0))
            cr = np.where(eq, np.exp(64.0 * v - MHN), 0.0)
            ps = np.where(eq, np.exp(64.0 * (u - 1.5) ** 2 - MHP), 0.0)
            CRv[rows] += cr.sum(axis=1)
            np.add.at(CRv, wcols, cr.sum(axis=0))
            PSv[rows] += ps.sum(axis=1)
            np.add.at(PSv, wcols, ps.sum(axis=0))

    # rows where dense-minus-CR cancellation is noise-dominated: recompute
    # their cross-class sum exactly in fp64 (cheap: [nbad, B] matmul)
    neg = NEG - CRv
    bad = neg < 0.05 * CRv
    if bad.any():
        idx = np.where(bad)[0]
        simb = e64[idx] @ e64.T
        Eb = np.exp(64.0 * (simb + 0.75) ** 2 - MHN)
        Eb[np.arange(len(idx)), idx] = 0.0
        sameb = ls[idx][:, None] == ls[None, :]
        neg[idx] = np.where(~sameb, Eb, 0.0).sum(axis=1)
    neg = np.maximum(neg, 1e-250)
    with np.errstate(divide="ignore", invalid="ignore"):
        negterm = np.log(neg) + MHN
        posterm = np.log(np.maximum(PSv, 1e-250)) + MHP
    per_row = np.logaddexp(0.0, negterm + posterm)
    per_row = np.where(valid_sorted, per_row, 0.0)
    count = int(valid_sorted.sum())
    return np.float32(per_row.sum() / max(count, 1))


def kernel(embeddings, labels, _trace=False):
    nc = _get_prog()
    in_maps, valid_sorted, ls, e64 = _prepare_inputs(embeddings, labels)
    res = run_bass_kernel_spmd(
        nc, in_maps, core_ids=list(range(NCORES)), trace=_trace
    )
    loss = _epilogue(res.results, valid_sorted, ls, e64)
    if _trace:
        return loss, res
    return loss


# revision 43
# speedup vs baseline: 1.0446x; 1.0446x over previous
"""CircleLossV2 on 8 Trainium2 NeuronCores (Bass/Tile) — symmetric triangle v10.

60.4us HW exec (vs 83.7us baseline), rel err 1.2e-6. Each group's first
512-col piece is prefetched into a dedicated PSUM bank at the end of the
previous group's step so the DVE never stalls at group boundaries.

Strategy (rebuilt from the 74us baseline):
  - Host: sort rows by label; per-core rotate by 512*k cols so each core owns
    tile-rows {0..3, 32..35} of its rotated copy (SPMD). sim is symmetric:
    tile-row i computes tiles [i, i+W) (W=33 for i<32 else 32) — every
    unordered pair once. Host normalizes in fp64, ships eT extended layout
    [128, 8704] bf16 (left pad 128, wrap).
  - Device per row-group: PE bf16 matmuls -> fp32 PSUM sim in 1536-col pieces
    (diag killed by a -2I matmul); ONE fused custom-DVE op per piece
    v=(s+0.75)^2 (sq(Src0+C0), registered via the framework's custom-DVE
    extension API) evacuates PSUM->SBUF fp32 in a single pass; a small
    ACT-Square share keeps DVE/ACT balanced; one big ACT Exp per group
    (bias -140, scale 64) -> E bf16 with accum_out row sums; column sums via
    one-hot-column matmuls into a single pre-zeroed [18,512] fp32 PSUM tile.
  - pw windows for the host pos/CR terms are v[:, 0:192] slices (fp32 u^2,
    exactly what the device exponentiates), DMA'd per group — no extra
    matmuls or copies.
  - No all_engine_barrier and no gpsimd anywhere: avoids serializing on the
    ~6us Q7 boot. Consts are framework-tracked tiles memset by the DVE.
    xt arrives as 4 large range-ordered dma_starts so compute starts ~2us in.
  - Host epilogue: row slots + flat colsums; right-sided same-class window
    (each unordered pair once, added to both members); fp64 recompute
    fallback for cancellation rows.
"""

import sys

sys.path.insert(0, "/opt/trn_rl_repo")

import numpy as np
from ml_dtypes import bfloat16

import concourse.bass as bass
import concourse.bacc as bacc
import concourse.mybir as mybir
import concourse.tile as tile
import concourse.dve_ops as dve_ops
from concourse.dve_spec import Spec, Src0, C0, sq, lower as dve_lower
from concourse.dve_spec import _has_src1 as dve_has_src1
from concourse.dve_uop import DveOpSpec
from concourse.bass_utils import run_bass_kernel_spmd

F32 = mybir.dt.float32
F16 = mybir.dt.float16
BF16 = mybir.dt.bfloat16
AF = mybir.ActivationFunctionType
OP = mybir.AluOpType

B = 8192
D = 128
NCORES = 8
EXT_OFF = 128
EXT_W = 8704
MHN = 140.0
MHP = 100.0
LOCAL_ROWS = [0, 1, 2, 3, 32, 33, 34, 35]
NSLOT = 13

_PROG = None


def _sqadd_op():
    """Register (once) and return the fused v=(s+c)^2 custom-DVE op."""
    name = "CIRCLE_SQADD_ANT"
    for o in dve_ops.OPS:
        if o.name == name:
            return o
    spec = Spec(
        body=sq(Src0 + C0),
        reference=lambda in0, in1, s0, s1, imm2: (
            (in0.astype(np.float32) + s0) ** 2
        ),
    )
    row = max(dve_ops._SUB_OPCODE_FOR_NAME.values()) + 1
    assert row < 0x20
    dve_ops._SUB_OPCODE_FOR_NAME[name] = row
    shas = {}
    for ver in ("v3",):
        u = dve_lower(spec, ver=ver)
        shas[ver] = DveOpSpec(
            name=name, opcode=row, uops=u, rd1_en=dve_has_src1(spec)
        ).sha(ver)
    op = dve_ops.DveOp(name, spec, subdim=False, uops_sha=shas)
    dve_ops.OPS.append(op)
    dve_ops.CUSTOM_DVE_SPECS[name] = spec
    return op


SQADD = _sqadd_op()


def _wtiles(i):
    return 33 if i < 32 else 32


def _build_schedule():
    groups = []
    for r, i in enumerate(LOCAL_ROWS):
        lo = EXT_OFF + i * 128
        wid = _wtiles(i) * 128
        # pieces: (piece_a, piece_b, act_tail_cols). Piece 0 is a small
        # 512-col prefetch piece computed in a dedicated PSUM bank at the
        # end of the PREVIOUS group's step, so the DVE never stalls at
        # group boundaries.
        pieces = [(0, 512, 0), (512, 1536, 0), (1536, 3072, 256),
                  (3072, wid, 0)]
        spans = []
        a, b = lo + 128, lo + wid
        while a < b:
            nb = min(b, 512 * (a // 512 + 1))
            spans.append((a // 512, a, nb))
            a = nb
        if r == 0:
            segs = [(0, 1536, 0), (1536, 3072, 8), (3072, wid, 10)]
        elif r == 7:
            segs = [(0, 1536, 7), (1536, 3072, 9), (3072, wid, 11)]
        else:
            segs = [(0, wid, r)]
        groups.append(dict(r=r, i=i, lo=lo, wid=wid, pieces=pieces,
                           spans=spans, segs=segs))
    return groups


GROUPS = _build_schedule()
NGRP = len(GROUPS)


def _build():
    nc = bacc.Bacc("TRN2", target_bir_lowering=False, debug=False,
                   num_devices=NCORES)

    xt_in = nc.dram_tensor("xt", [D, EXT_W], BF16, kind="ExternalInput")
    cst_in = nc.dram_tensor("cst", [128, 292], BF16, kind="ExternalInput")
    stats_out = nc.dram_tensor("stats", [128, NSLOT], F32, kind="ExternalOutput")
    cs_out = nc.dram_tensor("cs", [18, 512], F32, kind="ExternalOutput")
    pw_out = nc.dram_tensor("pw", [128, 1536], F32, kind="ExternalOutput")

    with tile.TileContext(nc) as tc:
        with (
            tc.tile_pool(name="cst", bufs=1) as cst,
            tc.tile_pool(name="sbv", bufs=3) as sbv,   # fp32 v = (s+0.75)^2
            tc.tile_pool(name="sbe", bufs=3) as sbe,   # bf16 E
            tc.tile_pool(name="psd", bufs=2, space="PSUM") as psd,
            tc.tile_pool(name="psx", bufs=1, space="PSUM") as psx,
            tc.tile_pool(name="psc", bufs=1, space="PSUM") as psc,
        ):
            # ---------------- inputs (range-ordered, few large DMAs) -------
            xt = cst.tile([128, EXT_W], BF16, tag="xt", name="xt")
            XT_CUTS = [0, 1664, 3584, 5760, EXT_W]
            nc.sync.dma_start(xt[:, 0:1664], xt_in.ap()[:, 0:1664])

            consts = cst.tile([128, 292], BF16, tag="consts", name="consts")
            nc.sync.dma_start(consts[:], cst_in.ap())
            i128 = consts[:, 0:128]
            n2i = consts[:, 128:256]
            onehot = consts[:, 256:291]  # rel col 17 is all-ones

            for c0, c1 in zip(XT_CUTS[1:-1], XT_CUTS[2:]):
                nc.sync.dma_start(xt[:, c0:c1], xt_in.ap()[:, c0:c1])

            # consts as framework-tracked tiles (no gpsimd, no barrier)
            cb = cst.tile([128, 2], F32, tag="cb", name="cb")
            nc.vector.memset(cb[:, 0:1], -MHN)
            nc.vector.memset(cb[:, 1:2], 0.75)
            nc.const_aps.aps[(F32, -MHN)] = cb[:, 0:1]
            nc.const_aps.aps[(F32, 0.75)] = cb[:, 1:2]

            NS = cst.tile([128, NSLOT], F32, tag="NS", name="NS")
            css = cst.tile([18, 512], F32, tag="css", name="css")

            # hoist ACT table loads to t=0
            warm0 = cst.tile([128, 1], F32, tag="warm0", name="warm0")
            nc.vector.memset(warm0[:], 0.0)
            warm1 = cst.tile([128, 1], F32, tag="warm1", name="warm1")
            nc.scalar.activation(warm1[:], warm0[:], AF.Exp,
                                 bias=-MHN, scale=64.0)
            nc.scalar.activation(warm1[:], warm0[:], AF.Square, bias=0.75)

            # pre-zero the colsum accumulator (all colsum matmuls accumulate)
            csP = psc.tile([18, 512], F32, tag="csP", name="csP")
            nc.vector.memset(csP[:], 0.0)

            state = {}

            def emit_cs_spans(gprev, spans, is_last_g):
                # a few colsum spans of an earlier group, interleaved between
                # main pieces so PE colsum bursts never block the psd refill
                E = state[("E", gprev["r"])]
                lo = gprev["lo"]
                nsp = len(gprev["spans"])
                for si, (h, aa, bb) in spans:
                    nc.tensor.matmul(
                        csP[0:18, aa - 512 * h: bb - 512 * h],
                        onehot[:, 17 - h: 35 - h],
                        E[:, aa - lo: bb - lo],
                        start=False,
                        stop=is_last_g and si == nsp - 1,
                        skip_group_check=True,
                    )

            def emit_piece(g, pi):
                lo, r = g["lo"], g["r"]
                pa, pb, act_w = g["pieces"][pi]
                w = pb - pa
                if pi == 0:
                    v = sbv.tile([128, 4224], F32, tag="v", name=f"v{r}")
                    state[("v", r)] = v
                v = state[("v", r)]
                if pi == 0:
                    ps_t = psx.tile([128, 512], F32, tag="px",
                                    name=f"px_{r}")
                else:
                    ps_t = psd.tile([128, 1536], F32, tag="ps",
                                    name=f"ps_{r}_{pa}")
                lhsT = xt[:, lo: lo + 128]
                for s0 in range(pa, pb, 512):
                    s1 = min(s0 + 512, pb)
                    has_diag = s0 == 0
                    nc.tensor.matmul(
                        ps_t[:, s0 - pa: s1 - pa], lhsT,
                        xt[:, lo + s0: lo + s1],
                        start=True, stop=not has_diag,
                    )
                    if has_diag:
                        nc.tensor.matmul(
                            ps_t[:, 0:128], n2i, i128,
                            start=False, stop=True, skip_group_check=True,
                        )
                dw = w - act_w
                nc.vector._custom_dve(
                    SQADD, out=v[:, pa:pa + dw], in0=ps_t[:, 0:dw], s0=0.75,
                )
                if act_w:
                    nc.scalar.activation(
                        v[:, pa + dw:pb], ps_t[:, dw:w], AF.Square, bias=0.75,
                    )
                if pi == 0:
                    # window slice for host pos/CR terms (v = (s+0.75)^2)
                    nc.sync.dma_start(
                        pw_out.ap()[:, r * 192:(r + 1) * 192], v[:, 0:192],
                    )

            def emit_exp(g):
                v = state.pop(("v", g["r"]))
                E = sbe.tile([128, 4224], BF16, tag="E", name=f"E{g['r']}")
                state[("E", g["r"])] = E
                for (a, b, slot) in g["segs"]:
                    nc.scalar.activation(
                        E[:, a:b], v[:, a:b], AF.Exp,
                        bias=-MHN, scale=64.0,
                        accum_out=NS[:, slot: slot + 1],
                    )

            emit_piece(GROUPS[0], 0)
            for step in range(NGRP + 2):
                g = GROUPS[step] if step < NGRP else None
                gprev = GROUPS[step - 2] if step >= 2 else None
                is_last_g = step - 2 == NGRP - 1
                # exp of the previous group FIRST on the ACT queue — its
                # input is already complete, while this step's A-share
                # square is not (avoids in-order head-of-line blocking)
                if 1 <= step < NGRP + 1:
                    emit_exp(GROUPS[step - 1])
                spans = (list(enumerate(gprev["spans"]))
                         if gprev is not None else [])
                third = (len(spans) + 1) // 2
                if g is not None:
                    for pi in range(1, len(g["pieces"])):
                        emit_piece(g, pi)
                        if pi >= 3 and spans:
                            take, spans = spans[:third], spans[third:]
                            emit_cs_spans(gprev, take, is_last_g)
                    if step + 1 < NGRP:
                        emit_piece(GROUPS[step + 1], 0)
                if spans:
                    emit_cs_spans(gprev, spans, is_last_g)
                if gprev is not None:
                    state.pop(("E", gprev["r"]))

            nc.sync.dma_start(stats_out.ap(), NS[:])
            nc.vector.tensor_copy(css[:], csP[:])
            nc.scalar.dma_start(cs_out.ap(), css[:])

    nc.compile()
    return nc


def _get_prog():
    global _PROG
    if _PROG is None:
        _PROG = _build()
    return _PROG


def _prepare_inputs(embeddings, labels):
    x = np.asarray(embeddings, dtype=np.float32)
    lab = np.asarray(labels)
    assert x.shape == (B, D) and lab.shape == (B,)

    perm = np.argsort(lab, kind="stable")
    xs = x[perm]
    ls = lab[perm]

    _, inv_idx, counts = np.unique(ls, return_inverse=True, return_counts=True)
    cnt_row = counts[inv_idx]
    valid_sorted = (cnt_row >= 2) & (B - cnt_row >= 1)
    assert counts.max() <= 64, "window of 192 requires class size <= 64"

    e64 = xs.astype(np.float64)
    e64 /= np.linalg.norm(e64, axis=1, keepdims=True)
    e = e64.astype(np.float32)
    eT = np.ascontiguousarray(e.T)

    cst = np.zeros((128, 292), dtype=bfloat16)
    cst[:, 0:128] = np.eye(128, dtype=bfloat16)
    cst[:, 128:256] = (-2.0 * np.eye(128)).astype(bfloat16)
    cst[:, 256 + 17] = 1.0

    ext_src = (np.arange(EXT_W) - EXT_OFF) % B
    in_maps = []
    for k in range(NCORES):
        sh = 512 * k
        rot_cols = (ext_src + sh) % B
        xt = np.ascontiguousarray(eT[:, rot_cols]).astype(bfloat16)
        in_maps.append({"xt": xt, "cst": cst})
    return in_maps, valid_sorted, ls, e64


def _epilogue(results, valid_sorted, ls, e64):
    NEG = np.zeros(B)
    CRv = np.zeros(B)
    PSv = np.zeros(B)
    prow = np.arange(128)
    ext_idx = np.arange(EXT_W) - EXT_OFF
    win = np.arange(192)
    triu = win[None, :] > prow[:, None]  # strict upper: each pair once
    for k in range(NCORES):
        st = np.asarray(results[k]["stats"], dtype=np.float64)
        cs = np.asarray(results[k]["cs"], dtype=np.float64)
        pwf = np.asarray(results[k]["pw"], dtype=np.float64)
        sh = 512 * k
        for g in GROUPS:
            rows = (sh + g["i"] * 128 + prow) % B
            for (_, _, slot) in g["segs"]:
                NEG[rows] += st[:, slot]
        csf = cs.reshape(-1)[:EXT_W]
        np.add.at(NEG, (ext_idx + sh) % B, csf)
        for r_idx, i in enumerate(LOCAL_ROWS):
            rows = (sh + i * 128 + prow) % B
            wcols = (sh + i * 128 + win) % B
            eq = (ls[rows][:, None] == ls[wcols][None, :]) & triu
            v = pwf[:, r_idx * 192:(r_idx + 1) * 192]  # (s+0.75)^2 fp32
            u = np.sqrt(np.maximum(v, 0.0))
            cr = np.where(eq, np.exp(64.0 * v - MHN), 0.0)
            ps = np.where(eq, np.exp(64.0 * (u - 1.5) ** 2 - MHP), 0.0)
            CRv[rows] += cr.sum(axis=1)
            np.add.at(CRv, wcols, cr.sum(axis=0))
            PSv[rows] += ps.sum(axis=1)
            np.add.at(PSv, wcols, ps.sum(axis=0))

    # rows where dense-minus-CR cancellation is noise-dominated: recompute
    # their cross-class sum exactly in fp64 (cheap: [nbad, B] matmul)
    neg = NEG - CRv
    bad = neg < 0.05 * CRv
    if bad.any():
        idx = np.where(bad)[0]
        simb = e64[idx] @ e64.T
        Eb = np.exp(64.0 * (simb + 0.75) ** 2 - MHN)
        Eb[np.arange(len(idx)), idx] = 0.0
        sameb = ls[idx][:, None] == ls[None, :]
        neg[idx] = np.where(~sameb, Eb, 0.0).sum(axis=1)
    neg = np.maximum(neg, 1e-250)
    with np.errstate(divide="ignore", invalid="ignore"):
        negterm = np.log(neg) + MHN
        posterm = np.log(np.maximum(PSv, 1e-250)) + MHP
    per_row = np.logaddexp(0.0, negterm + posterm)
    per_row = np.where(valid_sorted, per_row, 0.0)
    count = int(valid_sorted.sum())
    return np.float32(per_row.sum() / max(count, 1))


def kernel(embeddings, labels, _trace=False):
    nc = _get_prog()
    in_maps, valid_sorted, ls, e64 = _prepare_inputs(embeddings, labels)
    res = run_bass_kernel_spmd(
        nc, in_maps, core_ids=list(range(NCORES)), trace=_trace
    )
    loss = _epilogue(res.results, valid_sorted, ls, e64)
    if _trace:
        return loss, res
    return loss


# revision 44
# speedup vs baseline: 1.0625x; 1.0172x over previous
"""CircleLossV2 on 8 Trainium2 NeuronCores (Bass/Tile) — symmetric triangle v10.

60.4us HW exec (vs 83.7us baseline), rel err 1.2e-6. Each group's first
512-col piece is prefetched into a dedicated PSUM bank at the end of the
previous group's step so the DVE never stalls at group boundaries.

Strategy (rebuilt from the 74us baseline):
  - Host: sort rows by label; per-core rotate by 512*k cols so each core owns
    tile-rows {0..3, 32..35} of its rotated copy (SPMD). sim is symmetric:
    tile-row i computes tiles [i, i+W) (W=33 for i<32 else 32) — every
    unordered pair once. Host normalizes in fp64, ships eT extended layout
    [128, 8704] bf16 (left pad 128, wrap).
  - Device per row-group: PE bf16 matmuls -> fp32 PSUM sim in 1536-col pieces
    (diag killed by a -2I matmul); ONE fused custom-DVE op per piece
    v=(s+0.75)^2 (sq(Src0+C0), registered via the framework's custom-DVE
    extension API) evacuates PSUM->SBUF fp32 in a single pass; a small
    ACT-Square share keeps DVE/ACT balanced; one big ACT Exp per group
    (bias -140, scale 64) -> E bf16 with accum_out row sums; column sums via
    one-hot-column matmuls into a single pre-zeroed [18,512] fp32 PSUM tile.
  - pw windows for the host pos/CR terms are v[:, 0:192] slices (fp32 u^2,
    exactly what the device exponentiates), DMA'd per group — no extra
    matmuls or copies.
  - No all_engine_barrier and no gpsimd anywhere: avoids serializing on the
    ~6us Q7 boot. Consts are framework-tracked tiles memset by the DVE.
    xt arrives as 4 large range-ordered dma_starts so compute starts ~2us in.
  - Host epilogue: row slots + flat colsums; right-sided same-class window
    (each unordered pair once, added to both members); fp64 recompute
    fallback for cancellation rows.
"""

import sys

sys.path.insert(0, "/opt/trn_rl_repo")

import numpy as np
from ml_dtypes import bfloat16

import concourse.bass as bass
import concourse.bacc as bacc
import concourse.mybir as mybir
import concourse.tile as tile
import concourse.dve_ops as dve_ops
from concourse.dve_spec import Spec, Src0, C0, sq, lower as dve_lower
from concourse.dve_spec import _has_src1 as dve_has_src1
from concourse.dve_uop import DveOpSpec
from concourse.bass_utils import run_bass_kernel_spmd

F32 = mybir.dt.float32
F16 = mybir.dt.float16
BF16 = mybir.dt.bfloat16
AF = mybir.ActivationFunctionType
OP = mybir.AluOpType

B = 8192
D = 128
NCORES = 8
EXT_OFF = 128
EXT_W = 8704
MHN = 140.0
MHP = 100.0
LOCAL_ROWS = [0, 1, 2, 3, 32, 33, 34, 35]
NSLOT = 13

_PROG = None


def _sqadd_op():
    """Register (once) and return the fused v=(s+c)^2 custom-DVE op."""
    name = "CIRCLE_SQADD_ANT"
    for o in dve_ops.OPS:
        if o.name == name:
            return o
    spec = Spec(
        body=sq(Src0 + C0),
        reference=lambda in0, in1, s0, s1, imm2: (
            (in0.astype(np.float32) + s0) ** 2
        ),
    )
    row = max(dve_ops._SUB_OPCODE_FOR_NAME.values()) + 1
    assert row < 0x20
    dve_ops._SUB_OPCODE_FOR_NAME[name] = row
    shas = {}
    for ver in ("v3",):
        u = dve_lower(spec, ver=ver)
        shas[ver] = DveOpSpec(
            name=name, opcode=row, uops=u, rd1_en=dve_has_src1(spec)
        ).sha(ver)
    op = dve_ops.DveOp(name, spec, subdim=False, uops_sha=shas)
    dve_ops.OPS.append(op)
    dve_ops.CUSTOM_DVE_SPECS[name] = spec
    return op


SQADD = _sqadd_op()


def _wtiles(i):
    return 33 if i < 32 else 32


def _build_schedule():
    groups = []
    for r, i in enumerate(LOCAL_ROWS):
        lo = EXT_OFF + i * 128
        wid = _wtiles(i) * 128
        # pieces: (piece_a, piece_b, act_tail_cols). Piece 0 is a small
        # 512-col prefetch piece computed in a dedicated PSUM bank at the
        # end of the PREVIOUS group's step, so the DVE never stalls at
        # group boundaries.
        pieces = [(0, 512, 0), (512, 1536, 0), (1536, 3072, 256),
                  (3072, wid, 0)]
        spans = []
        a, b = lo + 128, lo + wid
        while a < b:
            nb = min(b, 512 * (a // 512 + 1))
            spans.append((a // 512, a, nb))
            a = nb
        if r == 0:
            segs = [(0, 1536, 0), (1536, 3072, 8), (3072, wid, 10)]
        elif r == 7:
            segs = [(0, 1536, 7), (1536, 3072, 9), (3072, wid, 11)]
        else:
            segs = [(0, wid, r)]
        groups.append(dict(r=r, i=i, lo=lo, wid=wid, pieces=pieces,
                           spans=spans, segs=segs))
    return groups


GROUPS = _build_schedule()
NGRP = len(GROUPS)


def _build():
    nc = bacc.Bacc("TRN2", target_bir_lowering=False, debug=False,
                   num_devices=NCORES)

    xt_in = nc.dram_tensor("xt", [D, EXT_W], BF16, kind="ExternalInput")
    cst_in = nc.dram_tensor("cst", [128, 292], BF16, kind="ExternalInput")
    stats_out = nc.dram_tensor("stats", [128, NSLOT], F32, kind="ExternalOutput")
    cs_out = nc.dram_tensor("cs", [18, 512], F32, kind="ExternalOutput")
    pw_out = nc.dram_tensor("pw", [128, 1536], F32, kind="ExternalOutput")

    with tile.TileContext(nc) as tc:
        with (
            tc.tile_pool(name="cst", bufs=1) as cst,
            tc.tile_pool(name="sbv", bufs=3) as sbv,   # fp32 v = (s+0.75)^2
            tc.tile_pool(name="sbe", bufs=3) as sbe,   # bf16 E
            tc.tile_pool(name="psd", bufs=2, space="PSUM") as psd,
            tc.tile_pool(name="psx", bufs=1, space="PSUM") as psx,
            tc.tile_pool(name="psc", bufs=1, space="PSUM") as psc,
        ):
            # ---------------- inputs (range-ordered, few large DMAs) -------
            xt = cst.tile([128, EXT_W], BF16, tag="xt", name="xt")
            XT_CUTS = [0, 1664, 3584, 5760, EXT_W]
            nc.sync.dma_start(xt[:, 0:1664], xt_in.ap()[:, 0:1664])

            consts = cst.tile([128, 292], BF16, tag="consts", name="consts")
            nc.sync.dma_start(consts[:], cst_in.ap())
            i128 = consts[:, 0:128]
            n2i = consts[:, 128:256]
            onehot = consts[:, 256:291]  # rel col 17 is all-ones

            for c0, c1 in zip(XT_CUTS[1:-1], XT_CUTS[2:]):
                nc.sync.dma_start(xt[:, c0:c1], xt_in.ap()[:, c0:c1])

            # consts as framework-tracked tiles (no gpsimd, no barrier)
            cb = cst.tile([128, 2], F32, tag="cb", name="cb")
            nc.vector.memset(cb[:, 0:1], -MHN)
            nc.vector.memset(cb[:, 1:2], 0.75)
            nc.const_aps.aps[(F32, -MHN)] = cb[:, 0:1]
            nc.const_aps.aps[(F32, 0.75)] = cb[:, 1:2]

            NS = cst.tile([128, NSLOT], F32, tag="NS", name="NS")
            css = cst.tile([18, 512], F32, tag="css", name="css")

            # hoist ACT table loads to t=0
            warm0 = cst.tile([128, 1], F32, tag="warm0", name="warm0")
            nc.vector.memset(warm0[:], 0.0)
            warm1 = cst.tile([128, 1], F32, tag="warm1", name="warm1")
            nc.scalar.activation(warm1[:], warm0[:], AF.Exp,
                                 bias=-MHN, scale=64.0)
            nc.scalar.activation(warm1[:], warm0[:], AF.Square, bias=0.75)

            # pre-zero the colsum accumulator (all colsum matmuls accumulate)
            csP = psc.tile([18, 512], F32, tag="csP", name="csP")
            nc.vector.memset(csP[:], 0.0)

            state = {}

            def emit_cs_spans(gprev, spans, is_last_g):
                # a few colsum spans of an earlier group, interleaved between
                # main pieces so PE colsum bursts never block the psd refill
                E = state[("E", gprev["r"])]
                lo = gprev["lo"]
                nsp = len(gprev["spans"])
                for si, (h, aa, bb) in spans:
                    nc.tensor.matmul(
                        csP[0:18, aa - 512 * h: bb - 512 * h],
                        onehot[:, 17 - h: 35 - h],
                        E[:, aa - lo: bb - lo],
                        start=False,
                        stop=is_last_g and si == nsp - 1,
                        skip_group_check=True,
                    )

            def emit_piece(g, pi):
                lo, r = g["lo"], g["r"]
                pa, pb, act_w = g["pieces"][pi]
                w = pb - pa
                if pi == 0:
                    v = sbv.tile([128, 4224], F32, tag="v", name=f"v{r}")
                    state[("v", r)] = v
                v = state[("v", r)]
                if pi == 0:
                    ps_t = psx.tile([128, 512], F32, tag="px",
                                    name=f"px_{r}")
                else:
                    ps_t = psd.tile([128, 1536], F32, tag="ps",
                                    name=f"ps_{r}_{pa}")
                lhsT = xt[:, lo: lo + 128]
                for s0 in range(pa, pb, 512):
                    s1 = min(s0 + 512, pb)
                    has_diag = s0 == 0
                    nc.tensor.matmul(
                        ps_t[:, s0 - pa: s1 - pa], lhsT,
                        xt[:, lo + s0: lo + s1],
                        start=True, stop=not has_diag,
                    )
                    if has_diag:
                        nc.tensor.matmul(
                            ps_t[:, 0:128], n2i, i128,
                            start=False, stop=True, skip_group_check=True,
                        )
                dw = w - act_w
                nc.vector._custom_dve(
                    SQADD, out=v[:, pa:pa + dw], in0=ps_t[:, 0:dw], s0=0.75,
                )
                if act_w:
                    nc.scalar.activation(
                        v[:, pa + dw:pb], ps_t[:, dw:w], AF.Square, bias=0.75,
                    )
                if pi == 0:
                    # window slice for host pos/CR terms (v = (s+0.75)^2)
                    nc.sync.dma_start(
                        pw_out.ap()[:, r * 192:(r + 1) * 192], v[:, 0:192],
                    )

            def emit_exp(g):
                v = state.pop(("v", g["r"]))
                E = sbe.tile([128, 4224], BF16, tag="E", name=f"E{g['r']}")
                state[("E", g["r"])] = E
                for (a, b, slot) in g["segs"]:
                    nc.scalar.activation(
                        E[:, a:b], v[:, a:b], AF.Exp,
                        bias=-MHN, scale=64.0,
                        accum_out=NS[:, slot: slot + 1],
                    )

            emit_piece(GROUPS[0], 0)
            for step in range(NGRP + 2):
                g = GROUPS[step] if step < NGRP else None
                gprev = GROUPS[step - 2] if step >= 2 else None
                is_last_g = step - 2 == NGRP - 1
                # exp of the previous group FIRST on the ACT queue — its
                # input is already complete, while this step's A-share
                # square is not (avoids in-order head-of-line blocking)
                if 1 <= step < NGRP + 1:
                    emit_exp(GROUPS[step - 1])
                spans = (list(enumerate(gprev["spans"]))
                         if gprev is not None else [])
                third = (len(spans) + 2) // 3
                if g is not None:
                    for pi in range(1, len(g["pieces"])):
                        emit_piece(g, pi)
                        if pi >= 2 and spans:
                            take, spans = spans[:third], spans[third:]
                            emit_cs_spans(gprev, take, is_last_g)
                    if step + 1 < NGRP:
                        emit_piece(GROUPS[step + 1], 0)
                if spans:
                    emit_cs_spans(gprev, spans, is_last_g)
                if gprev is not None:
                    state.pop(("E", gprev["r"]))

            nc.sync.dma_start(stats_out.ap(), NS[:])
            nc.vector.tensor_copy(css[:], csP[:])
            nc.scalar.dma_start(cs_out.ap(), css[:])

    nc.compile()
    return nc


def _get_prog():
    global _PROG
    if _PROG is None:
        _PROG = _build()
    return _PROG


def _prepare_inputs(embeddings, labels):
    x = np.asarray(embeddings, dtype=np.float32)
    lab = np.asarray(labels)
    assert x.shape == (B, D) and lab.shape == (B,)

    perm = np.argsort(lab, kind="stable")
    xs = x[perm]
    ls = lab[perm]

    _, inv_idx, counts = np.unique(ls, return_inverse=True, return_counts=True)
    cnt_row = counts[inv_idx]
    valid_sorted = (cnt_row >= 2) & (B - cnt_row >= 1)
    assert counts.max() <= 64, "window of 192 requires class size <= 64"

    e64 = xs.astype(np.float64)
    e64 /= np.linalg.norm(e64, axis=1, keepdims=True)
    e = e64.astype(np.float32)
    eT = np.ascontiguousarray(e.T)

    cst = np.zeros((128, 292), dtype=bfloat16)
    cst[:, 0:128] = np.eye(128, dtype=bfloat16)
    cst[:, 128:256] = (-2.0 * np.eye(128)).astype(bfloat16)
    cst[:, 256 + 17] = 1.0

    ext_src = (np.arange(EXT_W) - EXT_OFF) % B
    in_maps = []
    for k in range(NCORES):
        sh = 512 * k
        rot_cols = (ext_src + sh) % B
        xt = np.ascontiguousarray(eT[:, rot_cols]).astype(bfloat16)
        in_maps.append({"xt": xt, "cst": cst})
    return in_maps, valid_sorted, ls, e64


def _epilogue(results, valid_sorted, ls, e64):
    NEG = np.zeros(B)
    CRv = np.zeros(B)
    PSv = np.zeros(B)
    prow = np.arange(128)
    ext_idx = np.arange(EXT_W) - EXT_OFF
    win = np.arange(192)
    triu = win[None, :] > prow[:, None]  # strict upper: each pair once
    for k in range(NCORES):
        st = np.asarray(results[k]["stats"], dtype=np.float64)
        cs = np.asarray(results[k]["cs"], dtype=np.float64)
        pwf = np.asarray(results[k]["pw"], dtype=np.float64)
        sh = 512 * k
        for g in GROUPS:
            rows = (sh + g["i"] * 128 + prow) % B
            for (_, _, slot) in g["segs"]:
                NEG[rows] += st[:, slot]
        csf = cs.reshape(-1)[:EXT_W]
        np.add.at(NEG, (ext_idx + sh) % B, csf)
        for r_idx, i in enumerate(LOCAL_ROWS):
            rows = (sh + i * 128 + prow) % B
            wcols = (sh + i * 128 + win) % B
            eq = (ls[rows][:, None] == ls[wcols][None, :]) & triu
            v = pwf[:, r_idx * 192:(r_idx + 1) * 192]  # (s+0.75)^2 fp32
            u = np.sqrt(np.maximum(v, 0.0))
            cr = np.where(eq, np.exp(64.0 * v - MHN), 0.0)
            ps = np.where(eq, np.exp(64.0 * (u - 1.5) ** 2 - MHP), 0.0)
            CRv[rows] += cr.sum(axis=1)
            np.add.at(CRv, wcols, cr.sum(axis=0))
            PSv[rows] += ps.sum(axis=1)
            np.add.at(PSv, wcols, ps.sum(axis=0))

    # rows where dense-minus-CR cancellation is noise-dominated: recompute
    # their cross-class sum exactly in fp64 (cheap: [nbad, B] matmul)
    neg = NEG - CRv
    bad = neg < 0.05 * CRv
    if bad.any():
        idx = np.where(bad)[0]
        simb = e64[idx] @ e64.T
        Eb = np.exp(64.0 * (simb + 0.75) ** 2 - MHN)
        Eb[np.arange(len(idx)), idx] = 0.0
        sameb = ls[idx][:, None] == ls[None, :]
        neg[idx] = np.where(~sameb, Eb, 0.0).sum(axis=1)
    neg = np.maximum(neg, 1e-250)
    with np.errstate(divide="ignore", invalid="ignore"):
        negterm = np.log(neg) + MHN
        posterm = np.log(np.maximum(PSv, 1e-250)) + MHP
    per_row = np.logaddexp(0.0, negterm + posterm)
    per_row = np.where(valid_sorted, per_row, 0.0)
    count = int(valid_sorted.sum())
    return np.float32(per_row.sum() / max(count, 1))


def kernel(embeddings, labels, _trace=False):
    nc = _get_prog()
    in_maps, valid_sorted, ls, e64 = _prepare_inputs(embeddings, labels)
    res = run_bass_kernel_spmd(
        nc, in_maps, core_ids=list(range(NCORES)), trace=_trace
    )
    loss = _epilogue(res.results, valid_sorted, ls, e64)
    if _trace:
        return loss, res
    return loss
